# revision 37
# baseline (speedup 1.0000x reference)
"""Trainium2 Bass kernel for nn_App_Enc (attention pooling + weighted recombine).

Reference computation (per sample b):
    p        = softmax(raw_tps, axis=-1)                      # [N, S, S], per-row over w
    app_vec  = einsum('khw,nhw->nk', x, p)                    # [N, K]
    num      = einsum('nhw,nk->khw', fitted_cj, app_vec)      # [K, S, S]
    denom    = 1 + sum_n fitted_cj                            # [S, S]
    out      = num / denom                                    # [K, S, S]

Sharding: pure data parallel over batch B=16 -> 8 cores x 2 samples.
"""

import numpy as np

import concourse.bass as bass
import concourse.tile as tile
from concourse import mybir
from concourse.bass_utils import run_bass_kernel_spmd

# Problem constants (hardcoded per harness contract).
B = 16
N_HM = 32
K_APP = 16
S_FULL = 256
N_CORES = 8
BPC = B // N_CORES  # samples per core = 2

P = 128  # SBUF partitions
F32 = mybir.dt.float32
F32R = mybir.dt.float32r
BF16 = mybir.dt.bfloat16


def build_program(S=S_FULL, bpc=BPC, parts="full"):
    """Build the per-core Bass program. All 8 cores run the same graph on
    their own 2-sample shard; there is no cross-core communication."""
    HW = S * S
    WB = S // P           # w-blocks per row (2 for S=256)
    assert S % P == 0
    NROWS = N_HM * S      # (n,h) rows per sample
    RSUB = 4              # 128-row sub-tiles per raw super-tile
    NSUP = NROWS // (P * RSUB)
    CPS = HW // P         # hw chunks of 128 per sample (contraction tiles)
    XU = 8                # x sub-tiles per x super-tile
    XG = 8                # chunks packed per x sub-tile
    XSUP = CPS // (XU * XG)
    F2 = 512              # einsum-2 moving free dim (one PSUM bank of f32)
    CH2 = HW // F2        # einsum-2 hw chunks (each covers BOTH samples)
    GC = 8                # chunks per packed output group (4096 hw)
    NG = CH2 // GC        # output windows of 4096 hw
    NGH = NG // 2         # einsum-2 processes chunk-pairs (c, c + CH2/2)
    M2 = 2 * bpc * K_APP  # einsum-2 out rows: (chunk-half q, sample s, k) blocks

    nc = bass.Bass()

    x_ext = nc.declare_dram_parameter("x", [bpc, K_APP, S, S], F32, isOutput=False)
    raw_ext = nc.declare_dram_parameter("raw", [bpc, N_HM, S, S], F32, isOutput=False)
    fit_ext = nc.declare_dram_parameter("fit", [bpc, N_HM, S, S], F32, isOutput=False)
    idn_ext = nc.declare_dram_parameter("idn", [P, P], F32, isOutput=False)
    out_ext = nc.declare_dram_parameter("out", [bpc, K_APP, S, S], F32, isOutput=True)

    x_flat = [x_ext[b].rearrange("k h w -> k (h w)") for b in range(bpc)]
    raw_flat = [raw_ext[b].rearrange("n h w -> (n h) w") for b in range(bpc)]
    fit_flat = fit_ext[:].rearrange("s n h w -> s n (h w)")
    out_flat = out_ext[:].rearrange("s k h w -> s k (h w)")

    with tile.TileContext(nc) as tc:
        import contextlib

        ctx = contextlib.ExitStack()
        with ctx:
            singles = ctx.enter_context(tc.tile_pool(name="singles", bufs=1))
            xt_pool = ctx.enter_context(tc.tile_pool(name="xt", bufs=2))
            pt_pool = ctx.enter_context(tc.tile_pool(name="pt", bufs=3))
            av_pool = ctx.enter_context(tc.tile_pool(name="avs", bufs=2))
            raw_pool = ctx.enter_context(tc.tile_pool(name="raw", bufs=3))
            pe_pool = ctx.enter_context(tc.tile_pool(name="p_exp", bufs=3))
            ps_pool = ctx.enter_context(tc.tile_pool(name="p_scaled", bufs=3))
            sums_pool = ctx.enter_context(tc.tile_pool(name="sums", bufs=3))
            xs_pool = ctx.enter_context(tc.tile_pool(name="xs", bufs=3))
            xsb_pool = ctx.enter_context(tc.tile_pool(name="xsb", bufs=2))

            tp_psum = ctx.enter_context(
                tc.tile_pool(name="tp_psum", bufs=2, space="PSUM")
            )
            av_psum = ctx.enter_context(
                tc.tile_pool(name="av_psum", bufs=2, space="PSUM")
            )
            e2_psum = ctx.enter_context(
                tc.tile_pool(name="e2_psum", bufs=2, space="PSUM")
            )

            # --- identity (fp32 + bf16 copies), av_aug stationary ---
            idn_f32 = singles.tile([P, P], F32)
            nc.sync.dma_start(out=idn_f32[:], in_=idn_ext[:])
            idn_bf = singles.tile([P, P], BF16)
            nc.vector.tensor_copy(out=idn_bf[:], in_=idn_f32[:])

            zbias = singles.tile([P, 1], F32)
            nc.vector.memset(zbias[:], 0)
            # block-diagonal stationary for einsum-2: rows (q, s, n), cols (q, s, k)
            av_aug = singles.tile([2 * bpc * N_HM, M2], F32R)
            nc.vector.memset(av_aug[:].bitcast(F32), 0)
            # ones-block matrix: OB[r, m] = 1 iff r//N_HM == m//N_HM; used as the
            # D-matmul lhsT so each 32-row block of its output carries that
            # (q, s) block's denominator, already replicated across partitions
            oblk = singles.tile([2 * bpc * N_HM, 2 * bpc * N_HM], BF16)
            nc.vector.memset(oblk[:], 0)
            for b in range(2 * bpc):
                nc.vector.memset(
                    oblk[b * N_HM : (b + 1) * N_HM, b * N_HM : (b + 1) * N_HM], 1.0
                )

            xt_tiles = []
            for s in range(bpc):
                # ---------- phase 1: softmax -> p (bf16, normalized) ----------
                pt_tiles = [
                    pt_pool.tile([P, N_HM, S], BF16, tag="pt", name=f"pt_{s}_{wh}")
                    for wh in range(WB)
                ]
                for T in range(NSUP):
                    rtile = raw_pool.tile([P, RSUB, S], F32, tag="raw")
                    src = raw_flat[s][T * (P * RSUB) : (T + 1) * (P * RSUB), :]
                    nc.sync.dma_start(
                        out=rtile[:], in_=src.rearrange("(a p) w -> p a w", p=P)
                    )
                    petile = pe_pool.tile([P, RSUB, S], BF16, tag="pe")
                    rowsum = sums_pool.tile([P, RSUB], F32, tag="rowsum")
                    rz = sums_pool.tile([P, RSUB], F32, tag="rz")
                    for a in range(RSUB):
                        nc.scalar.activation(
                            out=petile[:, a, :],
                            in_=rtile[:, a, :],
                            func=mybir.ActivationFunctionType.Exp,
                            bias=zbias[:],
                            accum_out=rowsum[:, a : a + 1],
                        )
                    nc.vector.reciprocal(out=rz[:], in_=rowsum[:])
                    pstile = ps_pool.tile([P, RSUB, S], BF16, tag="ps")
                    for a in range(RSUB):
                        nc.vector.tensor_scalar_mul(
                            out=pstile[:, a, :],
                            in0=petile[:, a, :],
                            scalar1=rz[:, a : a + 1],
                        )
                    # ---------- phase 2: transpose p blocks ----------
                    # 128-row block `blk` holds rows of a single n: n = blk//(S/P),
                    # h-range (blk % (S/P))*P. In the [n, h]-flat free space of a
                    # pt tile, block blk lands at offset blk*P.
                    for a0 in range(0, RSUB, 2):
                        blk = RSUB * T + a0
                        for wh in range(WB):
                            tp = tp_psum.tile([P, 2, P], F32, tag="tp")
                            for i in range(2):
                                nc.tensor.matmul(
                                    out=tp[:, i, :],
                                    lhsT=pstile[:, a0 + i, wh * P : (wh + 1) * P],
                                    rhs=idn_bf[:],
                                    start=True,
                                    stop=True,
                                )
                            nc.vector.tensor_copy(
                                out=pt_tiles[wh][:].rearrange("p n h -> p (n h)")[
                                    :, blk * P : (blk + 2) * P
                                ],
                                in_=tp[:].rearrange("p a b -> p (a b)"),
                            )

                # ---------- phase 3: transpose x ----------
                # xs partitions are (k, g): k-major keeps the DMA source AP at
                # 3 dims ((u j) collapse into one contiguous run of 1024).
                # xt_all[p, T, g, u, k] holds x^T for chunk c = XU*XG*T + XG*g + u
                xt_all = xt_pool.tile([P, XSUP, XG, XU, K_APP], BF16, tag="xt")
                xt_tiles.append(xt_all)
                for T in range(XSUP):
                    xst = xs_pool.tile([P, XU, P], F32, tag="xs")
                    src = x_flat[s].rearrange(
                        "k (t g uj) -> t k g uj", t=XSUP, g=XG, uj=XU * P
                    )[T]
                    nc.gpsimd.dma_start(
                        out=xst[:].rearrange("p u j -> p (u j)"),
                        in_=src,
                    )
                    xsb = xsb_pool.tile([P, XU, P], BF16, tag="xsb")
                    nc.vector.tensor_copy(out=xsb[:], in_=xst[:])
                    for u0 in range(0, XU, 2):
                        tp = tp_psum.tile([P, 2, P], F32, tag="tp")
                        for i in range(2):
                            nc.tensor.matmul(
                                out=tp[:, i, :],
                                lhsT=xsb[:, u0 + i, :],
                                rhs=idn_bf[:],
                                start=True,
                                stop=True,
                            )
                        # psum cols iterate (i, k, g); scatter into xt_all
                        nc.vector.tensor_copy(
                            out=xt_all[:, T, :, u0 : u0 + 2, :].transpose(
                                [0, 2, 3, 1]
                            ),
                            in_=tp[:].rearrange("p a b -> p (a b)"),
                        )

                # ---------- phase 4: app_vec accumulation ----------
                if parts == "sm":
                    continue
                avp = av_psum.tile([K_APP, N_HM], F32, tag="av")
                n_mm = 0
                for wh in range(WB):
                    for h in range(S):
                        c = WB * h + wh
                        nc.tensor.matmul(
                            out=avp[:],
                            lhsT=xt_all[:, c // (XU * XG), (c % (XU * XG)) // XU, c % XU, :],
                            rhs=pt_tiles[wh][:, :, h],
                            start=(n_mm == 0),
                            stop=(n_mm == CPS - 1),
                        )
                        n_mm += 1
                # evacuate app_vec^T, transpose to [N, K], write into av_aug blocks
                avt_sb = av_pool.tile([K_APP, N_HM], F32, tag="avt")
                nc.vector.tensor_copy(out=avt_sb[:], in_=avp[:])
                av2 = av_psum.tile([N_HM, K_APP], F32, tag="av")
                nc.tensor.matmul(
                    out=av2[:],
                    lhsT=avt_sb[:],
                    rhs=idn_f32[0:K_APP, 0:K_APP],
                    start=True,
                    stop=True,
                )
                for q in range(2):
                    nc.vector.tensor_copy(
                        out=av_aug[(q * bpc + s) * N_HM : (q * bpc + s + 1) * N_HM,
                                   (q * bpc + s) * K_APP : (q * bpc + s + 1) * K_APP],
                        in_=av2[:],
                    )

            fit_pool = ctx.enter_context(tc.tile_pool(name="fit", bufs=2))
            ftr_pool = ctx.enter_context(tc.tile_pool(name="fitr", bufs=2))
            r_pool = ctx.enter_context(tc.tile_pool(name="rpool", bufs=3))
            fs_pool = ctx.enter_context(tc.tile_pool(name="fspool", bufs=3))
            sg_pool = ctx.enter_context(tc.tile_pool(name="sg", bufs=2))
            d_psum = ctx.enter_context(
                tc.tile_pool(name="d_psum", bufs=2, space="PSUM")
            )

            # ---------- phase 5: einsum-2 + denom + divide ----------
            if parts in ("sm", "sm_av"):
                ot0 = singles.tile([1, 16], F32)
                nc.vector.memset(ot0[:], 0)
                nc.sync.dma_start(out=out_flat[0, :, 0:1].transpose([1, 0]), in_=ot0[:])
                parts_skip_e2 = True
            else:
                parts_skip_e2 = False
            # Each matmul contracts 128 = (q, s, n) rows: two hw-chunks (c and
            # c + CH2/2) of both samples at once, via the block-diag stationary.
            av_r = av_aug[:]
            ob_r = oblk[:]
            fit_src = fit_flat.rearrange(
                "s n (q gg f) -> gg q s n f", q=2, f=GC * F2
            )
            HFC = GC // 2  # chunks per fitted half-tile
            for gh in range((2 * NGH) if not parts_skip_e2 else 0):
                g, hf = gh // 2, gh % 2
                ft = fit_pool.tile([2 * bpc * N_HM, HFC * F2], F32, tag="fit")
                for q in range(2):
                    # Pool/SWDGE issue stream: prefetches while SP is busy
                    # with the softmax-phase loads
                    nc.gpsimd.dma_start(
                        out=ft[q * bpc * N_HM : (q + 1) * bpc * N_HM, :],
                        in_=fit_src[g, q][:, :, hf * (HFC * F2) : (hf + 1) * (HFC * F2)],
                    )
                # bf16 copy (rounded) for the denominator matmul, on gpsimd
                ftr = ftr_pool.tile([2 * bpc * N_HM, HFC * F2], BF16, tag="ftr")
                nc.gpsimd.tensor_copy(out=ftr[:], in_=ft[:])
                stg = sg_pool.tile([M2, HFC * F2], F32, tag="stg")
                for cl in range(HFC):
                    cc = hf * HFC + cl
                    ftc = ft[:, cl * F2 : (cl + 1) * F2]
                    # D (replicated over each 32-row (q,s) block) via ones-block mm
                    dp = d_psum.tile([P, F2], F32, tag="dp")
                    nc.tensor.matmul(
                        out=dp[:], lhsT=ob_r,
                        rhs=ftr[:, cl * F2 : (cl + 1) * F2],
                        start=True, stop=True,
                    )
                    # R = 1 / (1 + D); evac folds the +1 into the copy bias
                    rt = r_pool.tile([P, F2], F32, tag="rt")
                    nc.scalar.activation(
                        out=rt[:], in_=dp[:],
                        func=mybir.ActivationFunctionType.Copy, bias=1.0,
                    )
                    nc.vector.reciprocal(out=rt[:], in_=rt[:])
                    # scale fitted by R, then the recombine matmul is pre-divided
                    fs = fs_pool.tile([P, F2], F32R, tag="fs")
                    nc.vector.tensor_mul(out=fs[:], in0=ftc, in1=rt[:])
                    ep = e2_psum.tile([M2, F2], F32, tag="e2")
                    nc.tensor.matmul(
                        out=ep[:], lhsT=av_r, rhs=fs[:],
                        start=True, stop=True,
                    )
                    nc.scalar.copy(
                        out=stg[:, cl * F2 : (cl + 1) * F2], in_=ep[:]
                    )
                for q in range(2):
                    w_g = g + q * NGH  # 4096-wide output hw window
                    off = w_g * (GC * F2) + hf * (HFC * F2)
                    nc.sync.dma_start(
                        out=out_flat[:, :, off : off + HFC * F2],
                        in_=stg[q * bpc * K_APP : (q + 1) * bpc * K_APP, :],
                    )

    return nc


# Walrus in this toolchain accepts at most ONE sync-wait on datapath
# instructions; hoist excess waits onto standalone sequencer EventSemaphore
# instructions (the same thing raw-bass wait_ge emits).
_SEQ_OPS = {"EventSemaphore", "Branch", "SemaphoreOp", "Call",
            "EventSemaphoreRangeClear", "PseudoSyncBarrier", "Halt", "Notify"}


def _legalize_sync_waits(d, max_waits=1):
    for fn in d["functions"]:
        for blk in fn["blocks"]:
            out = []
            for ins in blk["instructions"]:
                si = ins.get("sync_info")
                w = (si or {}).get("on_wait") or []
                if si and len(w) > max_waits and ins.get("opcode") not in _SEQ_OPS:
                    extra, keep = w[:-max_waits], w[-max_waits:]
                    for j, ew in enumerate(extra):
                        out.append({
                            "debug": ins.get("debug", 0),
                            "engine": ins["engine"],
                            "ins": [], "outs": [],
                            "name": f"{ins['name']}-esw{j}",
                            "opcode": "EventSemaphore",
                            "sync_info": {"on_update": [], "on_wait": [ew]},
                        })
                    si["on_wait"] = keep
                out.append(ins)
            blk["instructions"] = out
    return d


def _patch_serialization(nc):
    import json as _json

    orig = nc.to_json_bytes

    def patched():
        d = _json.loads(orig())
        _legalize_sync_waits(d)
        return _json.dumps(d).encode()

    nc.to_json_bytes = patched
    return nc


_CACHE = {}


def _get_program():
    key = (S_FULL, BPC)
    if key not in _CACHE:
        _CACHE[key] = _patch_serialization(build_program())
    return _CACHE[key]


_RUN_OPTS = {}  # test harness may set {"trace": True}
LAST_RESULT = None


def kernel(x, raw_tps, fitted_cj):
    global LAST_RESULT
    nc = _get_program()
    idn = np.eye(P, dtype=np.float32)
    in_maps = []
    for core in range(N_CORES):
        b0 = core * BPC
        in_maps.append(
            {
                "x": np.ascontiguousarray(x[b0 : b0 + BPC]),
                "raw": np.ascontiguousarray(raw_tps[b0 : b0 + BPC]),
                "fit": np.ascontiguousarray(fitted_cj[b0 : b0 + BPC]),
                "idn": idn,
            }
        )
    res = run_bass_kernel_spmd(
        nc, in_maps, core_ids=list(range(N_CORES)), **_RUN_OPTS
    )
    LAST_RESULT = res
    outs = [
        np.asarray(res.results[i]["out"]).reshape(BPC, K_APP, S_FULL, S_FULL)
        for i in range(N_CORES)
    ]
    return np.concatenate(outs, axis=0)


# revision 38
# speedup vs baseline: 1.6822x; 1.6822x over previous
"""Trainium2 Bass kernel for nn_App_Enc (attention pooling + weighted recombine).

Reference computation (per sample b):
    p        = softmax(raw_tps, axis=-1)                      # [N, S, S], per-row over w
    app_vec  = einsum('khw,nhw->nk', x, p)                    # [N, K]
    num      = einsum('nhw,nk->khw', fitted_cj, app_vec)      # [K, S, S]
    denom    = 1 + sum_n fitted_cj                            # [S, S]
    out      = num / denom                                    # [K, S, S]

Sharding: pure data parallel over batch B=16 -> 8 cores x 2 samples.
"""

import numpy as np

import concourse.bass as bass
import concourse.tile as tile
from concourse import mybir
from concourse.bass_utils import run_bass_kernel_spmd

# Problem constants (hardcoded per harness contract).
B = 16
N_HM = 32
K_APP = 16
S_FULL = 256
N_CORES = 8
BPC = B // N_CORES  # samples per core = 2

P = 128  # SBUF partitions
F32 = mybir.dt.float32
F32R = mybir.dt.float32r
BF16 = mybir.dt.bfloat16


def build_program(S=S_FULL, bpc=BPC, parts="full"):
    """Build the per-core Bass program. All 8 cores run the same graph on
    their own 2-sample shard; there is no cross-core communication."""
    HW = S * S
    WB = S // P           # w-blocks per row (2 for S=256)
    assert S % P == 0
    NROWS = N_HM * S      # (n,h) rows per sample
    RSUB = 4              # 128-row sub-tiles per raw super-tile
    NSUP = NROWS // (P * RSUB)
    CPS = HW // P         # hw chunks of 128 per sample (contraction tiles)
    XU = 8                # x sub-tiles per x super-tile
    XG = 8                # chunks packed per x sub-tile
    XSUP = CPS // (XU * XG)
    F2 = 512              # einsum-2 moving free dim (one PSUM bank of f32)
    CH2 = HW // F2        # einsum-2 hw chunks (each covers BOTH samples)
    GC = 8                # chunks per packed output group (4096 hw)
    NG = CH2 // GC        # output windows of 4096 hw
    NGH = NG // 2         # einsum-2 processes chunk-pairs (c, c + CH2/2)
    M2 = 2 * bpc * K_APP  # einsum-2 out rows: (chunk-half q, sample s, k) blocks

    nc = bass.Bass()

    x_ext = nc.declare_dram_parameter("x", [bpc, K_APP, S, S], F32, isOutput=False)
    raw_ext = nc.declare_dram_parameter("raw", [bpc, N_HM, S, S], F32, isOutput=False)
    fit_ext = nc.declare_dram_parameter("fit", [bpc, N_HM, S, S], F32, isOutput=False)
    idn_ext = nc.declare_dram_parameter("idn", [P, P], F32, isOutput=False)
    out_ext = nc.declare_dram_parameter("out", [bpc, K_APP, S, S], F32, isOutput=True)

    x_flat = [x_ext[b].rearrange("k h w -> k (h w)") for b in range(bpc)]
    raw_flat = [raw_ext[b].rearrange("n h w -> (n h) w") for b in range(bpc)]
    fit_flat = fit_ext[:].rearrange("s n h w -> s n (h w)")
    out_flat = out_ext[:].rearrange("s k h w -> s k (h w)")

    with tile.TileContext(nc) as tc:
        import contextlib

        ctx = contextlib.ExitStack()
        with ctx:
            singles = ctx.enter_context(tc.tile_pool(name="singles", bufs=1))
            xt_pool = ctx.enter_context(tc.tile_pool(name="xt", bufs=2))
            pt_pool = ctx.enter_context(tc.tile_pool(name="pt", bufs=3))
            av_pool = ctx.enter_context(tc.tile_pool(name="avs", bufs=2))
            raw_pool = ctx.enter_context(tc.tile_pool(name="raw", bufs=3))
            pe_pool = ctx.enter_context(tc.tile_pool(name="p_exp", bufs=3))
            ps_pool = ctx.enter_context(tc.tile_pool(name="p_scaled", bufs=3))
            sums_pool = ctx.enter_context(tc.tile_pool(name="sums", bufs=3))
            xs_pool = ctx.enter_context(tc.tile_pool(name="xs", bufs=3))
            xsb_pool = ctx.enter_context(tc.tile_pool(name="xsb", bufs=2))

            tp_psum = ctx.enter_context(
                tc.tile_pool(name="tp_psum", bufs=2, space="PSUM")
            )
            av_psum = ctx.enter_context(
                tc.tile_pool(name="av_psum", bufs=2, space="PSUM")
            )
            e2_psum = ctx.enter_context(
                tc.tile_pool(name="e2_psum", bufs=2, space="PSUM")
            )

            # --- identity (fp32 + bf16 copies), av_aug stationary ---
            idn_f32 = singles.tile([P, P], F32)
            nc.sync.dma_start(out=idn_f32[:], in_=idn_ext[:])
            idn_bf = singles.tile([P, P], BF16)
            nc.vector.tensor_copy(out=idn_bf[:], in_=idn_f32[:])

            zbias = singles.tile([P, 1], F32)
            nc.vector.memset(zbias[:], 0)
            # block-diagonal stationary for einsum-2: rows (q, s, n), cols (q, s, k)
            av_aug = singles.tile([2 * bpc * N_HM, M2], F32R)
            nc.vector.memset(av_aug[:].bitcast(F32), 0)
            # ones-block matrix: OB[r, m] = 1 iff r//N_HM == m//N_HM; used as the
            # D-matmul lhsT so each 32-row block of its output carries that
            # (q, s) block's denominator, already replicated across partitions
            oblk = singles.tile([2 * bpc * N_HM, 2 * bpc * N_HM], BF16)
            nc.vector.memset(oblk[:], 0)
            for b in range(2 * bpc):
                nc.vector.memset(
                    oblk[b * N_HM : (b + 1) * N_HM, b * N_HM : (b + 1) * N_HM], 1.0
                )

            xt_tiles = []
            for s in range(bpc):
                # ---------- phase 1: softmax -> p (bf16, normalized) ----------
                pt_tiles = [
                    pt_pool.tile([P, N_HM, S], BF16, tag="pt", name=f"pt_{s}_{wh}")
                    for wh in range(WB)
                ]
                # Row-pair packing: partition p of heatmap n holds rows
                # (2p, 2p+1) -> 2KB contiguous DRAM runs per descriptor.
                HP = S // 2   # partitions per heatmap block
                GH = 2        # heatmaps per raw tile
                raw_rp = raw_flat[s].rearrange("(n ph i) w -> ph n i w", ph=HP, i=2)
                for T in range(N_HM // GH):
                    base = T * GH
                    rtile = raw_pool.tile([HP, GH, 2, S], F32, tag="raw")
                    nc.sync.dma_start(
                        out=rtile[:], in_=raw_rp[:, base : base + GH]
                    )
                    petile = pe_pool.tile([HP, GH, 2, S], BF16, tag="pe")
                    rowsum = sums_pool.tile([HP, GH, 2], F32, tag="rowsum")
                    rz = sums_pool.tile([HP, GH, 2], F32, tag="rz")
                    for g in range(GH):
                        for i in range(2):
                            nc.scalar.activation(
                                out=petile[:, g, i, :],
                                in_=rtile[:, g, i, :],
                                func=mybir.ActivationFunctionType.Exp,
                                bias=zbias[:HP],
                                accum_out=rowsum[:, g, i : i + 1],
                            )
                    nc.vector.reciprocal(out=rz[:], in_=rowsum[:])
                    pstile = ps_pool.tile([HP, GH, 2, S], BF16, tag="ps")
                    for g in range(GH):
                        for i in range(2):
                            nc.vector.tensor_scalar_mul(
                                out=pstile[:, g, i, :],
                                in0=petile[:, g, i, :],
                                scalar1=rz[:, g, i : i + 1],
                            )
                    # ---------- phase 2: transpose p blocks ----------
                    # parity-slice i of heatmap n: transposed col j is h=2j+i,
                    # so the evac scatters with a stride-2 h destination.
                    for g in range(GH):
                        n_idx = base + g
                        for wh in range(WB):
                            tp = tp_psum.tile([P, 2, HP], F32, tag="tp")
                            for i in range(2):
                                nc.tensor.matmul(
                                    out=tp[:, i, :],
                                    lhsT=pstile[:, g, i, wh * P : (wh + 1) * P],
                                    rhs=idn_bf[:HP, :HP],
                                    start=True,
                                    stop=True,
                                )
                            nc.vector.tensor_copy(
                                out=bass.AP(
                                    tensor=pt_tiles[wh][:].tensor,
                                    offset=pt_tiles[wh][:].offset + n_idx * S,
                                    ap=[[N_HM * S, P], [1, 2], [2, HP]],
                                ),
                                in_=tp[:].rearrange("p a b -> p (a b)"),
                            )

                # ---------- phase 3: transpose x ----------
                # xs partitions are (k, g): k-major keeps the DMA source AP at
                # 3 dims ((u j) collapse into one contiguous run of 1024).
                # xt_all[p, T, g, u, k] holds x^T for chunk c = XU*XG*T + XG*g + u
                xt_all = xt_pool.tile([P, XSUP, XG, XU, K_APP], BF16, tag="xt")
                xt_tiles.append(xt_all)
                for T in range(XSUP):
                    xst = xs_pool.tile([P, XU, P], F32, tag="xs")
                    src = x_flat[s].rearrange(
                        "k (t g uj) -> t k g uj", t=XSUP, g=XG, uj=XU * P
                    )[T]
                    nc.gpsimd.dma_start(
                        out=xst[:].rearrange("p u j -> p (u j)"),
                        in_=src,
                    )
                    xsb = xsb_pool.tile([P, XU, P], BF16, tag="xsb")
                    nc.vector.tensor_copy(out=xsb[:], in_=xst[:])
                    for u0 in range(0, XU, 2):
                        tp = tp_psum.tile([P, 2, P], F32, tag="tp")
                        for i in range(2):
                            nc.tensor.matmul(
                                out=tp[:, i, :],
                                lhsT=xsb[:, u0 + i, :],
                                rhs=idn_bf[:],
                                start=True,
                                stop=True,
                            )
                        # psum cols iterate (i, k, g); scatter into xt_all
                        nc.vector.tensor_copy(
                            out=xt_all[:, T, :, u0 : u0 + 2, :].transpose(
                                [0, 2, 3, 1]
                            ),
                            in_=tp[:].rearrange("p a b -> p (a b)"),
                        )

                # ---------- phase 4: app_vec accumulation ----------
                if parts == "sm":
                    continue
                avp = av_psum.tile([K_APP, N_HM], F32, tag="av")
                n_mm = 0
                for wh in range(WB):
                    for h in range(S):
                        c = WB * h + wh
                        nc.tensor.matmul(
                            out=avp[:],
                            lhsT=xt_all[:, c // (XU * XG), (c % (XU * XG)) // XU, c % XU, :],
                            rhs=pt_tiles[wh][:, :, h],
                            start=(n_mm == 0),
                            stop=(n_mm == CPS - 1),
                        )
                        n_mm += 1
                # evacuate app_vec^T, transpose to [N, K], write into av_aug blocks
                avt_sb = av_pool.tile([K_APP, N_HM], F32, tag="avt")
                nc.vector.tensor_copy(out=avt_sb[:], in_=avp[:])
                av2 = av_psum.tile([N_HM, K_APP], F32, tag="av")
                nc.tensor.matmul(
                    out=av2[:],
                    lhsT=avt_sb[:],
                    rhs=idn_f32[0:K_APP, 0:K_APP],
                    start=True,
                    stop=True,
                )
                for q in range(2):
                    nc.vector.tensor_copy(
                        out=av_aug[(q * bpc + s) * N_HM : (q * bpc + s + 1) * N_HM,
                                   (q * bpc + s) * K_APP : (q * bpc + s + 1) * K_APP],
                        in_=av2[:],
                    )

            fit_pool = ctx.enter_context(tc.tile_pool(name="fit", bufs=2))
            ftr_pool = ctx.enter_context(tc.tile_pool(name="fitr", bufs=2))
            r_pool = ctx.enter_context(tc.tile_pool(name="rpool", bufs=3))
            fs_pool = ctx.enter_context(tc.tile_pool(name="fspool", bufs=3))
            sg_pool = ctx.enter_context(tc.tile_pool(name="sg", bufs=2))
            d_psum = ctx.enter_context(
                tc.tile_pool(name="d_psum", bufs=2, space="PSUM")
            )

            # ---------- phase 5: einsum-2 + denom + divide ----------
            if parts in ("sm", "sm_av"):
                ot0 = singles.tile([1, 16], F32)
                nc.vector.memset(ot0[:], 0)
                nc.sync.dma_start(out=out_flat[0, :, 0:1].transpose([1, 0]), in_=ot0[:])
                parts_skip_e2 = True
            else:
                parts_skip_e2 = False
            # Each matmul contracts 128 = (q, s, n) rows: two hw-chunks (c and
            # c + CH2/2) of both samples at once, via the block-diag stationary.
            av_r = av_aug[:]
            ob_r = oblk[:]
            fit_src = fit_flat.rearrange(
                "s n (q gg f) -> gg q s n f", q=2, f=GC * F2
            )
            HFC = GC // 2  # chunks per fitted half-tile
            for gh in range((2 * NGH) if not parts_skip_e2 else 0):
                g, hf = gh // 2, gh % 2
                ft = fit_pool.tile([2 * bpc * N_HM, HFC * F2], F32, tag="fit")
                for q in range(2):
                    # Pool/SWDGE issue stream: prefetches while SP is busy
                    # with the softmax-phase loads
                    nc.gpsimd.dma_start(
                        out=ft[q * bpc * N_HM : (q + 1) * bpc * N_HM, :],
                        in_=fit_src[g, q][:, :, hf * (HFC * F2) : (hf + 1) * (HFC * F2)],
                    )
                # bf16 copy (rounded) for the denominator matmul, on gpsimd
                ftr = ftr_pool.tile([2 * bpc * N_HM, HFC * F2], BF16, tag="ftr")
                nc.gpsimd.tensor_copy(out=ftr[:], in_=ft[:])
                stg = sg_pool.tile([M2, HFC * F2], F32, tag="stg")
                for cl in range(HFC):
                    cc = hf * HFC + cl
                    ftc = ft[:, cl * F2 : (cl + 1) * F2]
                    # D (replicated over each 32-row (q,s) block) via ones-block mm
                    dp = d_psum.tile([P, F2], F32, tag="dp")
                    nc.tensor.matmul(
                        out=dp[:], lhsT=ob_r,
                        rhs=ftr[:, cl * F2 : (cl + 1) * F2],
                        start=True, stop=True,
                    )
                    # R = 1 / (1 + D); evac folds the +1 into the copy bias
                    rt = r_pool.tile([P, F2], F32, tag="rt")
                    nc.scalar.activation(
                        out=rt[:], in_=dp[:],
                        func=mybir.ActivationFunctionType.Copy, bias=1.0,
                    )
                    nc.vector.reciprocal(out=rt[:], in_=rt[:])
                    # scale fitted by R, then the recombine matmul is pre-divided
                    fs = fs_pool.tile([P, F2], F32R, tag="fs")
                    nc.vector.tensor_mul(out=fs[:], in0=ftc, in1=rt[:])
                    ep = e2_psum.tile([M2, F2], F32, tag="e2")
                    nc.tensor.matmul(
                        out=ep[:], lhsT=av_r, rhs=fs[:],
                        start=True, stop=True,
                    )
                    nc.scalar.copy(
                        out=stg[:, cl * F2 : (cl + 1) * F2], in_=ep[:]
                    )
                for q in range(2):
                    w_g = g + q * NGH  # 4096-wide output hw window
                    off = w_g * (GC * F2) + hf * (HFC * F2)
                    nc.sync.dma_start(
                        out=out_flat[:, :, off : off + HFC * F2],
                        in_=stg[q * bpc * K_APP : (q + 1) * bpc * K_APP, :],
                    )

    return nc


# Walrus in this toolchain accepts at most ONE sync-wait on datapath
# instructions; hoist excess waits onto standalone sequencer EventSemaphore
# instructions (the same thing raw-bass wait_ge emits).
_SEQ_OPS = {"EventSemaphore", "Branch", "SemaphoreOp", "Call",
            "EventSemaphoreRangeClear", "PseudoSyncBarrier", "Halt", "Notify"}


def _legalize_sync_waits(d, max_waits=1):
    for fn in d["functions"]:
        for blk in fn["blocks"]:
            out = []
            for ins in blk["instructions"]:
                si = ins.get("sync_info")
                w = (si or {}).get("on_wait") or []
                if si and len(w) > max_waits and ins.get("opcode") not in _SEQ_OPS:
                    extra, keep = w[:-max_waits], w[-max_waits:]
                    for j, ew in enumerate(extra):
                        out.append({
                            "debug": ins.get("debug", 0),
                            "engine": ins["engine"],
                            "ins": [], "outs": [],
                            "name": f"{ins['name']}-esw{j}",
                            "opcode": "EventSemaphore",
                            "sync_info": {"on_update": [], "on_wait": [ew]},
                        })
                    si["on_wait"] = keep
                out.append(ins)
            blk["instructions"] = out
    return d


def _patch_serialization(nc):
    import json as _json

    orig = nc.to_json_bytes

    def patched():
        d = _json.loads(orig())
        _legalize_sync_waits(d)
        return _json.dumps(d).encode()

    nc.to_json_bytes = patched
    return nc


_CACHE = {}


def _get_program():
    key = (S_FULL, BPC)
    if key not in _CACHE:
        _CACHE[key] = _patch_serialization(build_program())
    return _CACHE[key]


_RUN_OPTS = {}  # test harness may set {"trace": True}
LAST_RESULT = None


def kernel(x, raw_tps, fitted_cj):
    global LAST_RESULT
    nc = _get_program()
    idn = np.eye(P, dtype=np.float32)
    in_maps = []
    for core in range(N_CORES):
        b0 = core * BPC
        in_maps.append(
            {
                "x": np.ascontiguousarray(x[b0 : b0 + BPC]),
                "raw": np.ascontiguousarray(raw_tps[b0 : b0 + BPC]),
                "fit": np.ascontiguousarray(fitted_cj[b0 : b0 + BPC]),
                "idn": idn,
            }
        )
    res = run_bass_kernel_spmd(
        nc, in_maps, core_ids=list(range(N_CORES)), **_RUN_OPTS
    )
    LAST_RESULT = res
    outs = [
        np.asarray(res.results[i]["out"]).reshape(BPC, K_APP, S_FULL, S_FULL)
        for i in range(N_CORES)
    ]
    return np.concatenate(outs, axis=0)
